# revision 15
# baseline (speedup 1.0000x reference)
"""Distributed Trainium2 kernel for nn_Attention_335007449342.

Head-parallel tensor parallelism over 8 NeuronCores with a
sequence-parallel switch before the output projection:
  - LoRA adapters are folded into the dense weights on the host
    (W_eff = W + lora2 @ lora1, exact by linearity); the attention
    scale 1/sqrt(HD) is folded into wq_eff; the tiny adapter K/V
    projections (which use the plain wk/wv per the reference) are
    computed on the host.
  - each core owns 4 heads (512 of 4096 qkv dims): computes its slice
    of Q/K/V, full attention for its heads (batch-0 Q/K/V written
    directly to SBUF, batch-1 via a DRAM roundtrip whose loads overlap
    batch-0 attention, staggered per head), and the gated adapter
    cross-attention.
  - per batch, attention outputs (out^T, dims x tokens) are exchanged
    with a small bf16 AllToAll (2 MB per core) so core c ends with all
    4096 dims for tokens [c*256,(c+1)*256) of the batch, then projects
    locally with the full wo_eff streamed from HBM in 4 MB column
    blocks; batch-0 projection interleaves with batch-1 attention.

All matmuls run in bf16 (fp32 PSUM accumulation); softmax in fp32 without
max-subtraction (scores are O(10), mask -1e9 underflows exp to 0).
"""

import math
import sys

sys.path.insert(0, "/opt/trn_rl_repo")

import numpy as np
import ml_dtypes

B, S, D, H, HD, AL, R = 2, 2048, 4096, 32, 128, 10, 16
NCORES = 8
HPC = H // NCORES          # 4 heads per core
LD = HPC * HD              # 512 local qkv dims per core
T = B * S                  # 4096 tokens
TB = 512                   # token block / query row-block
NTB = T // TB              # 8
KC = D // 128              # 32 contraction chunks over D
SKC = S // 128             # 16 key chunks per batch
TPC = S // NCORES          # 256 tokens per core per batch (a2a shard)
NQR = S // TB              # 4 query row-blocks per batch
SCALE = 1.0 / math.sqrt(HD)
BF16 = ml_dtypes.bfloat16

_CACHE = {}


def _mask_pattern(mask_np):
    """Per (qr query block of 512, kc key chunk of 128): classify the mask.
    keep=False when the whole block is ~-inf (softmax weight 0 -> skip),
    need_mask=True when the block has any nonzero mask value."""
    m = np.asarray(mask_np, np.float32)[0, 0]
    keep, need = [], []
    for qr in range(NQR):
        krow, nrow = [], []
        for kc in range(SKC):
            blk = m[qr * TB:(qr + 1) * TB, kc * 128:(kc + 1) * 128].T
            krow.append(not bool((blk <= -1e8).all()))
            nrow.append(bool((blk != 0.0).any()) and krow[-1])
        keep.append(tuple(krow))
        need.append(tuple(nrow))
    return tuple(keep), tuple(need)


def _build(keep=None, need_mask=None):
    import concourse.bass as bass
    import concourse.mybir as mybir
    import concourse.tile as tile
    from concourse import bacc
    from concourse.masks import make_identity

    f32 = mybir.dt.float32
    bf16 = mybir.dt.bfloat16
    AF = mybir.ActivationFunctionType
    if keep is None:
        keep = tuple((True,) * SKC for _ in range(NQR))
    if need_mask is None:
        need_mask = keep

    nc = bacc.Bacc(None, target_bir_lowering=False, debug=True)

    xt = nc.declare_dram_parameter("xt", [D, T], bf16, isOutput=False)
    wqkvt = nc.declare_dram_parameter("wqkvt", [D, 3 * LD], bf16, isOutput=False)
    wot = nc.declare_dram_parameter("wot", [D, D], bf16, isOutput=False)
    aktp = nc.declare_dram_parameter("aktp", [128, HPC * B * AL], bf16, isOutput=False)
    avp = nc.declare_dram_parameter("avp", [B * AL, LD], bf16, isOutput=False)
    maskt = nc.declare_dram_parameter("maskt", [S, S], bf16, isOutput=False)
    gfac = nc.declare_dram_parameter("gfac", [128, HPC], f32, isOutput=False)
    out = nc.declare_dram_parameter("out", [B * TPC, D], f32, isOutput=True)

    rg8 = [list(range(NCORES))]
    # batch-0 runs h-major (head h's last use is unit 4h+3, so batch-1's
    # per-head reloads of the shared K/Q/V tiles can start early);
    # batch-1 runs qr-major (order is free there)
    units_h = [(qr, h) for h in range(HPC) for qr in range(NQR)]
    units_q = [(qr, h) for qr in range(NQR) for h in range(HPC)]

    with tile.TileContext(nc) as tc:
        with tc.tile_pool(name="dram", bufs=1, space="DRAM") as dram, \
             tc.tile_pool(name="persist", bufs=1) as persist:
            qt_d = dram.tile([LD, S], bf16)          # batch-1 roundtrip
            kt_d = dram.tile([LD, S], bf16)
            v_d = dram.tile([S, LD], bf16)
            a2a_in0 = dram.tile([NCORES * LD, TPC], bf16)
            a2a_out0 = dram.tile([NCORES * LD, TPC], bf16)
            # batch-1 exchanged in two half-sequence collectives so the
            # second one's latency overlaps the start of its projection
            HTK = TPC // 2          # 128 tokens per core per half
            a2a_in1 = [dram.tile([NCORES * LD, HTK], bf16, name=f"a2ain1{x}")
                       for x in range(2)]
            a2a_out1 = [dram.tile([NCORES * LD, HTK], bf16, name=f"a2aout1{x}")
                        for x in range(2)]

            ident = persist.tile([128, 128], f32)
            make_identity(nc, ident)
            gfacsb = persist.tile([128, HPC], f32)
            nc.sync.dma_start(gfacsb, gfac[:])
            aktsb = persist.tile([128, HPC, B * AL], bf16)
            nc.sync.dma_start(aktsb, aktp[:].rearrange("p (m a) -> p m a", m=HPC))
            avsb = [persist.tile([AL, LD], bf16, name=f"avsb{b}") for b in range(B)]
            for b in range(B):
                nc.sync.dma_start(avsb[b], avp[b * AL:(b + 1) * AL, :])

            # mask chunks (shared by both batches): slots computed here,
            # tile allocated after phase 1 (SBUF is tight during QKV)
            needed = [(qr, k) for qr in range(NQR) for k in range(SKC)
                      if need_mask[qr][k]]
            mslot = {(qrk): i for i, qrk in enumerate(needed)}

            with tc.tile_pool(name="kqv", bufs=1) as kqv:
                ktsb = kqv.tile([128, HPC, S], bf16)
                qsb = kqv.tile([128, HPC, S], bf16)
                vasb = kqv.tile([128, SKC, HPC, HD + 1], bf16)
                nc.vector.memset(vasb[:, :, :, HD:HD + 1], 1.0)

                # ---------------- Phase 1: QKV projections ----------------
                with tc.tile_pool(name="wpool", bufs=1) as wpool, \
                     tc.tile_pool(name="xpool", bufs=2) as xpool, \
                     tc.tile_pool(name="spool", bufs=2) as spool, \
                     tc.tile_pool(name="qkps", bufs=4, space="PSUM") as qkps:
                    wsb = wpool.tile([128, KC, 3 * LD], bf16)
                    wre = wqkvt[:].rearrange("(kc p) m -> p kc m", p=128)
                    for kg in range(8):
                        eng = nc.sync if kg % 2 == 0 else nc.scalar
                        eng.dma_start(wsb[:, kg * 4:(kg + 1) * 4, :],
                                      wre[:, kg * 4:(kg + 1) * 4, :])

                    HT = TB // 2             # 256-token half blocks
                    for tb in range(NTB):
                        b = tb // NQR
                        for half in range(2):
                            ts = (tb % NQR) * TB + half * HT
                            t0 = tb * TB + half * HT
                            xsb = xpool.tile([128, KC, HT], bf16, tag="x")
                            nc.gpsimd.dma_start(
                                xsb,
                                xt[:].rearrange("(kc p) t -> p kc t", p=128)[
                                    :, :, t0:t0 + HT])
                            # q^T and k^T tiles ([outdim, tok])
                            for m in range(2 * HPC):
                                ps = qkps.tile([128, HT], f32, tag="ps")
                                for k in range(KC):
                                    nc.tensor.matmul(
                                        ps, wsb[:, k, m * 128:(m + 1) * 128],
                                        xsb[:, k, :],
                                        start=(k == 0), stop=(k == KC - 1))
                                h = m % HPC
                                if b == 0:
                                    dst = qsb if m < HPC else ktsb
                                    if m % 2 == 0:
                                        nc.scalar.activation(
                                            dst[:, h, ts:ts + HT], ps, AF.Copy)
                                    else:
                                        nc.vector.tensor_copy(
                                            dst[:, h, ts:ts + HT], ps)
                                else:
                                    osb = spool.tile([128, HT], bf16, tag="qk")
                                    if m % 2 == 0:
                                        nc.scalar.activation(osb, ps, AF.Copy)
                                    else:
                                        nc.vector.tensor_copy(osb, ps)
                                    dst_d = qt_d if m < HPC else kt_d
                                    nc.sync.dma_start(
                                        dst_d[h * 128:(h + 1) * 128, ts:ts + HT],
                                        osb)
                            # v tiles in [token, dim] layout
                            for tt in range(HT // 128):
                                kc = (tb % NQR) * (TB // 128) + half * 2 + tt
                                ps = qkps.tile([128, LD], f32, tag="pv")
                                for k in range(KC):
                                    nc.tensor.matmul(
                                        ps, xsb[:, k, tt * 128:(tt + 1) * 128],
                                        wsb[:, k, 2 * LD:3 * LD],
                                        start=(k == 0), stop=(k == KC - 1))
                                if b == 0:
                                    for h in range(HPC):
                                        if h % 2 == 0:
                                            nc.vector.tensor_copy(
                                                vasb[:, kc, h, :HD],
                                                ps[:, h * 128:(h + 1) * 128])
                                        else:
                                            nc.scalar.activation(
                                                vasb[:, kc, h, :HD],
                                                ps[:, h * 128:(h + 1) * 128],
                                                AF.Copy)
                                else:
                                    vsb = spool.tile([128, LD], bf16, tag="qk")
                                    if tt % 2 == 0:
                                        nc.vector.tensor_copy(vsb, ps)
                                    else:
                                        nc.scalar.activation(vsb, ps, AF.Copy)
                                    nc.sync.dma_start(
                                        v_d[ts + tt * 128:ts + (tt + 1) * 128, :],
                                        vsb)

                # mask chunks loaded after phase 1 (SBUF tight during QKV);
                # qr=0 slots first so unit 0's mask-add is ready earliest
                maskpool = tc.tile_pool(name="maskpool", bufs=1)
                mpersist = maskpool.__enter__()
                msb = None
                if 0 < len(needed) <= 16:
                    msb = mpersist.tile([128, len(needed), TB], bf16)
                    mre = maskt[:].rearrange("(kc p) q -> p kc q", p=128)
                    for i, (qr, k) in enumerate(needed):
                        nc.sync.dma_start(msb[:, i, :],
                                          mre[:, k, qr * TB:(qr + 1) * TB])

                def load_b1_head(h):
                    nc.sync.dma_start(ktsb[:, h, :], kt_d[h * 128:(h + 1) * 128, :])
                    nc.sync.dma_start(qsb[:, h, :], qt_d[h * 128:(h + 1) * 128, :])
                    nc.sync.dma_start(
                        vasb[:, :, h, :HD],
                        v_d[:, h * 128:(h + 1) * 128]
                        .rearrange("(kc p) d -> p kc d", p=128))

                def attn_batch(b, units, sps, ops, tps2, aps,
                               ppool, apool, zpool, mpool, interleave=None):
                    """Emit attention for one batch, 2-deep software pipeline.
                    interleave(i) emits extra work after unit i's PV block."""
                    mfull = None
                    if msb is None and any(any(r) for r in need_mask):
                        mfull = mpool.tile([128, SKC, TB], bf16, tag="mask")

                    def mchunk(qr, k):
                        if msb is not None:
                            return msb[:, mslot[(qr, k)], :]
                        return mfull[:, k, :]

                    def scores_block(u):
                        qr, h = u
                        kept = [k for k in range(SKC) if keep[qr][k]]
                        if msb is None and mfull is not None:
                            mre = maskt[:].rearrange("(kc p) q -> p kc q", p=128)
                            for k in kept:
                                if need_mask[qr][k]:
                                    nc.sync.dma_start(
                                        mfull[:, k, :],
                                        mre[:, k, qr * TB:(qr + 1) * TB])
                        qv = qsb[:, h, qr * TB:(qr + 1) * TB]
                        ptsb = ppool.tile([128, SKC, TB], bf16, tag="p")
                        for i2 in range(0, len(kept), 2):
                            pair = kept[i2:i2 + 2]
                            pss = sps.tile([128, 2, TB], f32, tag="s")
                            for j, k in enumerate(pair):
                                nc.tensor.matmul(
                                    pss[:, j, :],
                                    ktsb[:, h, k * 128:(k + 1) * 128],
                                    qv, start=True, stop=True)
                                if need_mask[qr][k]:
                                    nc.vector.tensor_add(
                                        pss[:, j, :], pss[:, j, :], mchunk(qr, k))
                            if len(pair) == 2 and pair[1] == pair[0] + 1:
                                nc.scalar.activation(
                                    ptsb[:, pair[0]:pair[0] + 2, :], pss, AF.Exp)
                            else:
                                for j, k in enumerate(pair):
                                    nc.scalar.activation(
                                        ptsb[:, k, :], pss[:, j, :], AF.Exp)
                        return ptsb

                    def adapter_block(u):
                        qr, h = u
                        qv = qsb[:, h, qr * TB:(qr + 1) * TB]
                        ats = []
                        for qs in range(TB // 128):
                            pa = aps.tile([128, AL], f32, tag="a")
                            nc.tensor.matmul(
                                pa, qv[:, qs * 128:(qs + 1) * 128],
                                aktsb[:, h, b * AL:(b + 1) * AL],
                                start=True, stop=True)
                            ae = apool.tile([128, AL], bf16, tag="ae")
                            sa = apool.tile([128, 1], f32, tag="sa")
                            nc.scalar.activation(ae, pa, AF.Exp, accum_out=sa)
                            ra = apool.tile([128, 1], f32, tag="ra")
                            nc.vector.reciprocal(ra, sa)
                            rg = apool.tile([128, 1], f32, tag="rg")
                            nc.vector.tensor_mul(rg, ra, gfacsb[:, h:h + 1])
                            asc = apool.tile([128, AL], f32, tag="asc")
                            nc.scalar.activation(asc, ae, AF.Copy, scale=rg)
                            pat = aps.tile([AL, 128], f32, tag="a")
                            nc.tensor.matmul(pat, asc, ident,
                                             is_transpose=True,
                                             start=True, stop=True)
                            atsb = apool.tile([AL, 128], bf16, tag=f"at{qs}")
                            nc.vector.tensor_copy(atsb, pat)
                            ats.append(atsb)
                        return ats

                    def pv_block(u, ptsb, ats):
                        qr, h = u
                        kept = [k for k in range(SKC) if keep[qr][k]]
                        pt2 = tps2.tile([128, TB], f32, tag="t2")
                        for qs in range(TB // 128):
                            po = ops.tile([128, HD + 1], f32, tag="o",
                                          padded_shape=[128, TB])
                            for i, k in enumerate(kept):
                                nc.tensor.matmul(
                                    po, ptsb[:, k, qs * 128:(qs + 1) * 128],
                                    vasb[:, k, h, :],
                                    start=(i == 0), stop=(i == len(kept) - 1))
                            rec = zpool.tile([128, 1], f32, tag="rec")
                            nc.vector.reciprocal(rec, po[:, HD:HD + 1])
                            osb = zpool.tile([128, HD], f32, tag="osb")
                            nc.vector.tensor_scalar_mul(osb, po[:, :HD], rec)
                            nc.tensor.matmul(
                                pt2[:, qs * 128:(qs + 1) * 128], osb, ident,
                                is_transpose=True, start=True, stop=False)
                            nc.tensor.matmul(
                                pt2[:, qs * 128:(qs + 1) * 128],
                                avsb[b][:, h * 128:(h + 1) * 128],
                                ats[qs], start=False, stop=True)
                        p2s = zpool.tile([128, TB], bf16, tag="p2s")
                        if h % 2 == 0:
                            nc.scalar.activation(p2s, pt2, AF.Copy)
                        else:
                            nc.vector.tensor_copy(p2s, pt2)
                        if b == 0:
                            for half in range(2):
                                j = 2 * qr + half
                                nc.gpsimd.dma_start(
                                    a2a_in0[j * LD + h * 128:
                                            j * LD + (h + 1) * 128, :],
                                    p2s[:, half * TPC:(half + 1) * TPC])
                        else:
                            dst = a2a_in1[qr // 2]
                            for qq in range(4):
                                j = (qr % 2) * 4 + qq
                                nc.gpsimd.dma_start(
                                    dst[j * LD + h * 128:
                                        j * LD + (h + 1) * 128, :],
                                    p2s[:, qq * 128:(qq + 1) * 128])

                    prev = None
                    for i, u in enumerate(units):
                        cur = (u, scores_block(u), adapter_block(u))
                        if prev is not None:
                            pv_block(*prev)
                            if interleave is not None:
                                interleave(i - 1)
                        prev = cur
                    pv_block(*prev)
                    if interleave is not None:
                        interleave(len(units) - 1)
                    if b == 0:
                        nc.gpsimd.collective_compute(
                            "AllToAll", bass.mybir.AluOpType.bypass,
                            replica_groups=rg8,
                            ins=[a2a_in0[:].opt()],
                            outs=[a2a_out0[:].opt()])

                # ------------- Phase 2: attention batch 0 + a2a0 -----------
                with tc.tile_pool(name="sps", bufs=2, space="PSUM") as sps, \
                     tc.tile_pool(name="ops", bufs=2, space="PSUM") as ops, \
                     tc.tile_pool(name="tps2", bufs=1, space="PSUM") as tps2, \
                     tc.tile_pool(name="aps", bufs=1, space="PSUM") as aps, \
                     tc.tile_pool(name="ppool", bufs=2) as ppool, \
                     tc.tile_pool(name="apool", bufs=2) as apool, \
                     tc.tile_pool(name="zpool", bufs=2) as zpool, \
                     tc.tile_pool(name="mpool", bufs=1) as mpool:

                    def inter0(i):
                        if i % NQR == NQR - 1:
                            load_b1_head(i // NQR)

                    attn_batch(0, units_h, sps, ops, tps2, aps,
                               ppool, apool, zpool, mpool, interleave=inter0)

                # -------- Phase 3: attention batch 1 + projection batch 0 ---
                with tc.tile_pool(name="wopool", bufs=2) as wopool, \
                     tc.tile_pool(name="atpool", bufs=1) as atpool, \
                     tc.tile_pool(name="opool", bufs=2) as opool:
                    wore = wot[:].rearrange("(kc p) o -> p kc o", p=128)

                    def load_att(src, ntok, name):
                        att = atpool.tile([128, KC, ntok], bf16, tag=name,
                                          name=name)
                        are = src[:].rearrange("(kc p) t -> p kc t", p=128)
                        for kg in range(4):
                            nc.sync.dma_start(att[:, kg * 8:(kg + 1) * 8, :],
                                              are[:, kg * 8:(kg + 1) * 8, :])
                        return att

                    def proj_chain(b, pps, attsb, tblk, oc, wosb):
                        pp = pps.tile([128, TB], f32, tag="pp")
                        for k in range(KC):
                            nc.tensor.matmul(
                                pp, attsb[:, k, :],
                                wosb[:, k, :],
                                start=(k == 0), stop=(k == KC - 1))
                        psb = opool.tile([128, TB], f32, tag="ps")
                        if (oc + tblk) % 2 == 0:
                            nc.scalar.activation(psb, pp, AF.Copy)
                        else:
                            nc.vector.tensor_copy(psb, pp)
                        nc.sync.dma_start(
                            out[b * TPC + tblk * 128:b * TPC + (tblk + 1) * 128,
                                oc * TB:(oc + 1) * TB], psb)

                    def proj_b0_oc(pps, oc, box):
                        if box[0] is None:
                            box[0] = load_att(a2a_out0, TPC, "attsb0")
                        wosb = wopool.tile([128, KC, TB], bf16, tag="wo")
                        nc.sync.dma_start(wosb, wore[:, :, oc * TB:(oc + 1) * TB])
                        for tblk in range(TPC // 128):
                            proj_chain(0, pps, box[0][:, :, tblk * 128:
                                                      (tblk + 1) * 128],
                                       tblk, oc, wosb)

                    with tc.tile_pool(name="sps1", bufs=1, space="PSUM") as sps1, \
                         tc.tile_pool(name="ops1", bufs=2, space="PSUM") as ops1, \
                         tc.tile_pool(name="tps21", bufs=1, space="PSUM") as tps21, \
                         tc.tile_pool(name="aps1", bufs=1, space="PSUM") as aps1, \
                         tc.tile_pool(name="pps0", bufs=2, space="PSUM") as pps0, \
                         tc.tile_pool(name="ppool1", bufs=2) as ppool1, \
                         tc.tile_pool(name="apool1", bufs=2) as apool1, \
                         tc.tile_pool(name="zpool1", bufs=2) as zpool1, \
                         tc.tile_pool(name="mpool1", bufs=1) as mpool1:
                        box0 = [None]

                        def inter1(i):
                            # a2a halves fire as soon as their tokens exist
                            # (qr-major: units 0-7 = first half); proj-b0 oc
                            # blocks at units 3,5,..,15 (oc7 after the loop)
                            if i == 7:
                                nc.gpsimd.collective_compute(
                                    "AllToAll", bass.mybir.AluOpType.bypass,
                                    replica_groups=rg8,
                                    ins=[a2a_in1[0][:].opt()],
                                    outs=[a2a_out1[0][:].opt()])
                            if i % 2 == 1 and i >= 3:
                                proj_b0_oc(pps0, (i - 3) // 2, box0)

                        attn_batch(1, units_q, sps1, ops1, tps21, aps1,
                                   ppool1, apool1, zpool1, mpool1,
                                   interleave=inter1)
                        nc.gpsimd.collective_compute(
                            "AllToAll", bass.mybir.AluOpType.bypass,
                            replica_groups=rg8,
                            ins=[a2a_in1[1][:].opt()],
                            outs=[a2a_out1[1][:].opt()])
                        proj_b0_oc(pps0, 7, box0)

                    # ---------------- Phase 4: projection batch 1 ----------
                    # paired oc blocks: both tblk0 chains (first-half tokens,
                    # attsb1a ready since mid-attention) run before the tblk1
                    # chains, covering the second collective's latency
                    with tc.tile_pool(name="pps1", bufs=4, space="PSUM") as pps1:
                        att1 = [None, None]
                        wtiles = {}
                        for p in range(4):
                            for oc in (2 * p, 2 * p + 1):
                                if att1[0] is None:
                                    att1[0] = load_att(a2a_out1[0], HTK,
                                                       "attsb1a")
                                wosb = wopool.tile([128, KC, TB], bf16,
                                                   tag="wo", name=f"wo1_{oc}")
                                nc.sync.dma_start(
                                    wosb, wore[:, :, oc * TB:(oc + 1) * TB])
                                wtiles[oc] = wosb
                                proj_chain(1, pps1, att1[0], 0, oc, wosb)
                            for oc in (2 * p, 2 * p + 1):
                                if att1[1] is None:
                                    att1[1] = load_att(a2a_out1[1], HTK,
                                                       "attsb1b")
                                proj_chain(1, pps1, att1[1], 1, oc,
                                           wtiles[oc])
                maskpool.__exit__(None, None, None)

    nc.compile()
    return nc


def _prep_inputs(x, mask, adapter, wq, wk, wv, wo,
                 lora_q1, lora_q2, lora_k1, lora_k2, lora_v1, lora_v2,
                 lora_o1, lora_o2, gate, new_gate):
    """Host-side sharding: returns in_maps (list of 8 dicts)."""
    def bf(a):
        return np.ascontiguousarray(np.asarray(a, np.float32).astype(BF16))

    f32 = np.float32
    wq_eff = (np.asarray(wq, f32)
              + np.asarray(lora_q2, f32) @ np.asarray(lora_q1, f32)) * SCALE
    wk_eff = np.asarray(wk, f32) + np.asarray(lora_k2, f32) @ np.asarray(lora_k1, f32)
    wv_eff = np.asarray(wv, f32) + np.asarray(lora_v2, f32) @ np.asarray(lora_v1, f32)
    wo_eff = np.asarray(wo, f32) + np.asarray(lora_o2, f32) @ np.asarray(lora_o1, f32)

    x2 = np.asarray(x, f32).reshape(T, D)
    xt = bf(x2.T)
    wot = bf(wo_eff.T)
    maskt = bf(np.asarray(mask, f32)[0, 0].T)
    gf_all = (np.tanh(np.asarray(gate, f32)[0, :, 0, 0])
              * np.asarray(new_gate, f32)[0, 0, 0, 0])

    # adapter K/V with the plain wk/wv (reference applies no LoRA there);
    # adapter scores use the pre-scaled q, so no extra scale needed here
    a2 = np.asarray(adapter, f32)                       # [B, AL, D]
    ak_all = a2 @ np.asarray(wk, f32).T                 # [B, AL, D]
    av_all = a2 @ np.asarray(wv, f32).T

    in_maps = []
    for c in range(NCORES):
        sl = slice(c * LD, (c + 1) * LD)
        wqkvt = bf(np.concatenate([wq_eff[sl].T, wk_eff[sl].T, wv_eff[sl].T],
                                  axis=1))
        akt_np = np.zeros((128, HPC, B, AL), f32)
        for m in range(HPC):
            for b in range(B):
                akt_np[:, m, b, :] = ak_all[b, :, c * LD + m * 128:
                                            c * LD + (m + 1) * 128].T
        aktp = bf(akt_np.reshape(128, HPC * B * AL))
        avp = bf(av_all[:, :, sl].reshape(B * AL, LD))
        gfac = np.tile(gf_all[c * HPC:(c + 1) * HPC][None, :],
                       (128, 1)).astype(f32)
        in_maps.append({
            "xt": xt, "wqkvt": wqkvt, "wot": wot, "aktp": aktp,
            "avp": avp, "maskt": maskt, "gfac": gfac,
        })
    return in_maps


def kernel(x, start_pos, freqs_cis, mask, adapter,
           wq, wk, wv, wo,
           lora_q1, lora_q2, lora_k1, lora_k2,
           lora_v1, lora_v2, lora_o1, lora_o2,
           gate, new_gate, _trace=False):
    from concourse.bass_utils import run_bass_kernel_spmd

    keep, need = _mask_pattern(mask)
    if _CACHE.get("pattern") != (keep, need):
        _CACHE["nc"] = _build(keep, need)
        _CACHE["pattern"] = (keep, need)
    nc = _CACHE["nc"]

    in_maps = _prep_inputs(x, mask, adapter, wq, wk, wv, wo,
                           lora_q1, lora_q2, lora_k1, lora_k2,
                           lora_v1, lora_v2, lora_o1, lora_o2, gate, new_gate)
    kw = {}
    if _trace:
        kw["tmpdir"] = "/tmp/ktrace"
        import os
        import shutil
        shutil.rmtree("/tmp/ktrace", ignore_errors=True)
        os.makedirs("/tmp/ktrace", exist_ok=True)
    res = run_bass_kernel_spmd(nc, in_maps, list(range(NCORES)), trace=_trace, **kw)
    _CACHE["last_exec_ns"] = res.exec_time_ns
    _CACHE["last_res"] = res
    outs = [np.asarray(res.results[c]["out"], np.float32) for c in range(NCORES)]
    # batch 0: core c rows [0:256] = tokens [c*256:(c+1)*256]
    # batch 1: rows [256:384] = tokens [c*128:(c+1)*128] (first half),
    #          rows [384:512] = tokens [1024+c*128:1024+(c+1)*128]
    full = np.empty((B, S, D), np.float32)
    for c, o in enumerate(outs):
        full[0, c * TPC:(c + 1) * TPC] = o[0:TPC]
        full[1, c * 128:(c + 1) * 128] = o[TPC:TPC + 128]
        full[1, S // 2 + c * 128:S // 2 + (c + 1) * 128] = o[TPC + 128:TPC + 256]
    return full
